# revision 7
# baseline (speedup 1.0000x reference)
"""DenseKANLayer Trainium2 kernel.

Math: for each edge e=(o,i), the reference computes a cubic B-spline
s_e(x) = sum_g c_basis[e,g] * B_{e,g}(x) on the 15-point knot row of e,
then y[b,o] = sum_i c_spl[o,i]*s_(o,i)(x[b,i]) + c_res[o,i]*silu(x[b,i]) + bias[o].

A cubic B-spline combination is exactly a sum of truncated powers:
s_e(x) = sum_m beta[e,m] * relu(x - t_m)^3, where beta_m is the jump of
s''' at knot m divided by 6 (computed host-side from knots + c_basis via
the exact B-spline derivative recurrence).  Features with t_m >= max(x)
vanish, so only the first F (typically 11) knots matter.  Folding
c_spl into beta gives one dense matmul:

    y.T = W @ Phi,  W[o, (m,i)] = c_spl[o,i]*beta[o*Ni+i, m]  (+ c_res chunk)
    Phi[(m,i), b]  = relu(x[b,i] - t[i,m])^3                  (+ silu chunk)

Device work per core (batch sharded 8 ways, raw bacc + manual semaphores):
- DMA1 (SP):  pack [x.T shard | -knots | bias] (128 x 76)
- DMA2 (ACT): W chunks [silu, f0..f5]   (128 x 896)
- DMA3 (SP):  W chunks [f6..f10]        (128 x 640)
- DVE: fused (x + (-t_m)) max 0 per feature, squares+cubes (wide, 2 groups)
- ACT: silu (single activation-table load, placed after the W DMA gen)
- PE:  12 accumulated fp32 matmuls (128x128 @ 128x64) into PSUM
- DVE: bias add PSUM -> SBUF ; SP: DMA out (128 x 64)

(A SWDGE prepare_only + trigger_dma scatter output was tried to hide the
output-DMA descriptor-gen latency; it double-executed on real hardware,
so the output uses a plain HWDGE DMA.)
"""

import numpy as np

N_IN, N_OUT, SPLINE_K, G = 128, 128, 3, 8
BATCH = 512
EDGES = N_IN * N_OUT
N_KNOTS = G + 2 * SPLINE_K + 1          # 15
N_COEF = G + SPLINE_K                   # 11
N_CORES = 8
BSHARD = BATCH // N_CORES               # 64

_COMPILED = {}


def _beta_from_bspline(knots, c_basis):
    """Truncated-power coefficients beta (EDGES, N_KNOTS) such that
    sum_g c[e,g] B_{e,g}(x) == sum_m beta[e,m] relu(x - knots[e,m])^3
    exactly (computed in float64)."""
    E = knots.shape[0]
    t = knots.astype(np.float64)
    c = c_basis.astype(np.float64)

    def deriv(c, k):
        m = c.shape[1]
        cpad = np.concatenate(
            [np.zeros((E, 1)), c, np.zeros((E, 1))], axis=1)
        g = np.arange(m + 1)
        denom = t[:, g + k] - t[:, g]
        with np.errstate(divide="ignore", invalid="ignore"):
            d = k * (cpad[:, 1:] - cpad[:, :-1]) / denom
        return np.where(denom == 0, 0.0, d)

    c3 = deriv(deriv(deriv(c, 3), 2), 1)          # s''' per interval (E, 14)
    c3pad = np.concatenate([np.zeros((E, 1)), c3, np.zeros((E, 1))], axis=1)
    return (c3pad[:, 1:] - c3pad[:, :-1]) / 6.0   # (E, 15)


def _reference_numpy(x, knots, c_basis, c_spl, c_res, bias):
    """Exact (slow) fallback for inputs the factorized kernel can't
    handle (knot rows differing across the n_out axis)."""
    batch = x.shape[0]
    x_ext = np.broadcast_to(x[:, None, :], (batch, N_OUT, N_IN)).reshape(batch, EDGES).T
    grid = knots[:, :, None]
    x_in = x_ext[:, None, :]
    b = ((x_in >= grid[:, :-1]) & (x_in < grid[:, 1:])).astype(np.float32)
    for order in range(1, SPLINE_K + 1):
        n0 = grid[:, order:-1] - grid[:, :-(order + 1)]
        n1 = grid[:, order + 1:] - grid[:, 1:-order]
        with np.errstate(divide="ignore", invalid="ignore"):
            left = np.where(n0 == 0, 0.0, (x_in - grid[:, :-(order + 1)]) / n0)
            right = np.where(n1 == 0, 0.0, (grid[:, order + 1:] - x_in) / n1)
        b = left * b[:, :-1] + right * b[:, 1:]
    spl = np.einsum("eg,egb->eb", c_basis, b).T
    y = c_spl.reshape(1, EDGES) * spl
    sig = 1.0 / (1.0 + np.exp(-x_ext.T))
    y = y + c_res.reshape(1, EDGES) * (x_ext.T * sig)
    return (y.reshape(batch, N_OUT, N_IN).sum(axis=2) + bias).astype(np.float32)


def _build_program(n_feat, w1ch=7):
    """Raw-bacc per-core program; n_feat truncated-power features + silu."""
    import concourse.bass as bass
    import concourse.mybir as mybir
    from concourse import bacc
    from concourse.hw_specs import get_activation_tables

    F = n_feat
    S = BSHARD
    NCH = F + 1
    w1ch = max(1, min(w1ch, NCH - 1))
    PCOLS = S + F + 1
    dt = mybir.dt.float32
    ACTF = mybir.ActivationFunctionType
    ALU = mybir.AluOpType

    class SlimBlock(bass.BassBlock):
        """Skip the exit drain + all-engine barrier; completion is carried
        by the explicit semaphore chain ending in s_y."""
        def __exit__(self, exc_type, exc_val, exc_tb):
            if exc_type is None:
                for engine, last_body in self.last_body.items():
                    with self.bass.body(last_body, parent=self.bass.cur_bb,
                                        allow_existing_parent=True):
                        engine.br(self.end_bb)
                self.bass.switch_bb(self.end_bb)

    class FastBacc(bacc.Bacc):
        """Skip the constructor's const-AP entry barrier: nothing reads the
        const tiles before a much-later semaphore wait."""
        _skip_entry_barrier = True

        def all_engine_barrier(self, **kw):
            if getattr(self, "_skip_entry_barrier", False):
                return
            return super().all_engine_barrier(**kw)

    nc = FastBacc("TRN2", target_bir_lowering=False, debug=False)
    nc._skip_entry_barrier = False

    xp = nc.dram_tensor("xp", [128, PCOLS], dt, kind="ExternalInput")
    w = nc.dram_tensor("w", [N_IN, NCH * N_OUT], dt, kind="ExternalInput")
    y = nc.dram_tensor("y", [N_OUT, S], dt, kind="ExternalOutput")

    with (
        nc.sbuf_tensor([128, PCOLS], dt) as XP,
        nc.sbuf_tensor([N_IN, NCH * N_OUT], dt) as WT,
        nc.sbuf_tensor([N_IN, NCH * S], dt) as PHI,
        nc.sbuf_tensor([N_IN, F * S], dt) as SQ,
        nc.sbuf_tensor([N_OUT, S], dt) as Y,
        nc.psum_tensor([N_OUT, S], dt) as acc,
        nc.semaphore("s_pack") as s_pack,
        nc.semaphore("s_w1") as s_w1,
        nc.semaphore("s_w2") as s_w2,
        nc.semaphore("s_phi0") as s_phi0,
        nc.semaphore("s_cub1") as s_cub1,
        nc.semaphore("s_cub2") as s_cub2,
        nc.semaphore("s_mm") as s_mm,
        nc.semaphore("s_bias") as s_bias,
        nc.semaphore("s_y") as s_y,
        SlimBlock(nc, "main") as block,
    ):
        X = XP[:, :S]
        NT = XP[:, S:S + F]
        BO = XP[:, S + F:S + F + 1]
        G1 = w1ch - 1

        @block.sync
        def _(sp):
            sp.dma_start(out=XP[:], in_=xp.ap()).then_inc(s_pack, 16)
            sp.dma_start(out=WT[:, w1ch * N_OUT:],
                         in_=w.ap()[:, w1ch * N_OUT:]).then_inc(s_w2, 16)
            sp.wait_ge(s_bias, 1)
            sp.dma_start(out=y.ap(), in_=Y[:]).then_inc(s_y, 16)
            sp.wait_ge(s_y, 16)

        @block.scalar
        def _(act):
            act.dma_start(out=WT[:, :w1ch * N_OUT],
                          in_=w.ap()[:, :w1ch * N_OUT]).then_inc(s_w1, 16)
            tabs = get_activation_tables(nc.m.arch)
            set_id = list(tabs).index("silu_and_others")
            ld = mybir.InstLoadActFuncSet(
                name=nc.get_next_instruction_name(), ins=[], outs=[],
                act_func_set_id=set_id)
            ld.engine = mybir.EngineType.Activation
            nc.scalar.add_instruction(ld)
            act.wait_ge(s_pack, 16)
            nc.scalar.activation(PHI[:, :S], X, ACTF.Silu).then_inc(s_phi0, 1)

        @block.vector
        def _(dve):
            dve.wait_ge(s_pack, 16)
            g1 = G1 * S
            for m in range(G1):
                nc.vector.tensor_scalar(
                    PHI[:, (1 + m) * S:(2 + m) * S], X, NT[:, m:m + 1], 0.0,
                    op0=ALU.add, op1=ALU.max)
            src1 = PHI[:, S:S + g1]
            nc.vector.tensor_mul(SQ[:, :g1], src1, src1)
            nc.vector.tensor_mul(src1, SQ[:, :g1], src1).then_inc(s_cub1, 1)
            for m in range(G1, F):
                nc.vector.tensor_scalar(
                    PHI[:, (1 + m) * S:(2 + m) * S], X, NT[:, m:m + 1], 0.0,
                    op0=ALU.add, op1=ALU.max)
            src2 = PHI[:, S + g1:NCH * S]
            nc.vector.tensor_mul(SQ[:, g1:F * S], src2, src2)
            nc.vector.tensor_mul(src2, SQ[:, g1:F * S], src2).then_inc(s_cub2, 1)
            dve.wait_ge(s_mm, 1)
            nc.vector.tensor_scalar_add(Y[:], acc[:], BO).then_inc(s_bias, 1)

        @block.tensor
        def _(pe):
            pe.wait_ge(s_w1, 16)
            pe.wait_ge(s_phi0, 1)
            nc.tensor.matmul(acc[:], WT[:, :N_OUT], PHI[:, :S],
                             start=True, stop=False)
            pe.wait_ge(s_cub1, 1)
            for m in range(1, w1ch):
                nc.tensor.matmul(acc[:], WT[:, m * N_OUT:(m + 1) * N_OUT],
                                 PHI[:, m * S:(m + 1) * S],
                                 start=False, stop=False)
            pe.wait_ge(s_w2, 16)
            pe.wait_ge(s_cub2, 1)
            for m in range(w1ch, NCH):
                mm = nc.tensor.matmul(acc[:], WT[:, m * N_OUT:(m + 1) * N_OUT],
                                      PHI[:, m * S:(m + 1) * S], start=False,
                                      stop=(m == NCH - 1))
            mm.then_inc(s_mm, 1)

    nc.compile()
    return nc


def kernel(x, knots, c_basis, c_spl, c_res, bias):
    x = np.asarray(x, np.float32)
    knots = np.asarray(knots, np.float32)
    c_basis = np.asarray(c_basis, np.float32)
    c_spl = np.asarray(c_spl, np.float32)
    c_res = np.asarray(c_res, np.float32)
    bias = np.asarray(bias, np.float32)

    # Factorization requires the knot row to be shared across the n_out
    # axis for each input column i (true for the reference's broadcast
    # knots).  Otherwise fall back to the exact host implementation.
    kr = knots.reshape(N_OUT, N_IN, N_KNOTS)
    t_col = kr[0]                                     # (N_IN, N_KNOTS)
    if not np.array_equal(kr, np.broadcast_to(t_col[None], kr.shape)):
        return _reference_numpy(x, knots, c_basis, c_spl, c_res, bias)
    # Truncated powers don't vanish past the last knot (where the
    # reference's B-spline support ends), so x beyond it needs the
    # exact path.
    if np.any(x >= t_col[:, -1][None, :]):
        return _reference_numpy(x, knots, c_basis, c_spl, c_res, bias)

    beta = _beta_from_bspline(knots, c_basis)         # (EDGES, 15) f64

    # Features m with t[i,m] >= max_b x[b,i] for every i contribute
    # nothing (relu always 0); drop them.  11 for the reference setup.
    xmax = x.max(axis=0)                              # (N_IN,)
    active = (t_col.T < xmax[None, :]).any(axis=1)    # (N_KNOTS,)
    n_feat = int(np.nonzero(active)[0].max() + 1) if active.any() else 1
    # Extra (always-zero) features are harmless; keep F large enough for
    # the fixed W1/W2 chunk split in the device program.
    n_feat = min(max(n_feat, 8), N_KNOTS)

    betaR = beta.reshape(N_OUT, N_IN, N_KNOTS)[:, :, :n_feat]
    wk = (c_spl[:, :, None].astype(np.float64) * betaR).astype(np.float32)
    # chunk order: [silu, f0 .. f{n_feat-1}]
    chunks = [c_res.T.astype(np.float32)]
    for m in range(n_feat):
        chunks.append(np.ascontiguousarray(wk[:, :, m].T))
    w_host = np.ascontiguousarray(
        np.concatenate(chunks, axis=1), dtype=np.float32)  # (128, (F+1)*128)
    nt_host = np.ascontiguousarray(-t_col[:, :n_feat], dtype=np.float32)
    bo_host = np.ascontiguousarray(bias[:, None], dtype=np.float32)
    xT = np.ascontiguousarray(x.T, dtype=np.float32)       # (N_IN, BATCH)

    if n_feat not in _COMPILED:
        _COMPILED[n_feat] = _build_program(n_feat)
    nc = _COMPILED[n_feat]

    from concourse.bass_utils import run_bass_kernel_spmd
    core_ids = list(range(N_CORES))
    in_maps = []
    for c in core_ids:
        pack = np.concatenate(
            [xT[:, c * BSHARD:(c + 1) * BSHARD], nt_host, bo_host], axis=1)
        in_maps.append({"xp": np.ascontiguousarray(pack), "w": w_host})
    res = run_bass_kernel_spmd(nc, in_maps, core_ids)
    y_oT = np.concatenate([res.results[c]["y"] for c in core_ids], axis=1)
    return np.ascontiguousarray(y_oT.T, dtype=np.float32)
